# revision 1
# baseline (speedup 1.0000x reference)
"""Trainium2 Bass kernel for CustomConv2d:

    out[b,o,h,w] = sum_{c,i,j} xpad[b,c,h+i,w+j]^exponent[o,c,i,j] * weight[o,c,i,j] + bias[o]

Shapes (hardcoded): x [16,32,64,64] f32, weight/exponent [32,32,3,3] f32,
bias [32] f32, K=3, stride=1, pad=1 -> out [16,32,64,64] f32.

Algorithm
---------
The per-tap power is the only thing separating this from a standard conv.
Since x in (0,1) and e in [0,1), write x^e = exp(e * ln x) and interpolate
in the *exponent* direction at R Chebyshev nodes e_k on [0,1]:

    x^e  ~=  sum_k  l_k(e) * x^{e_k}          (l_k = Lagrange basis)

which turns the whole op into R standard 3x3 convolutions of the feature
maps F_k = x^{e_k} (one ScalarE exp pass each) against repacked weights
A_k = weight * l_k(exponent) (computed on-device on the VectorE).
R=8 nodes give a worst-case per-term error ~1.5e-2 concentrated at the
rare smallest-x elements; end-to-end output error is dominated by the
bf16 matmul operands either way (measured rel err ~2.8e-3 for both R=8
and R=12, vs a 2e-2 gate), so R=8 is used for 1/3 less TensorE work.

Distribution: pure data-parallel over batch, 2 batches per each of the
8 NeuronCores; weight/exponent/bias are replicated (they're tiny).

Per-core kernel:
  1. DMA x-shard replicated 4x across partition groups -> xrep[128, 2*64*64]
     (partition = (replica g, cin c)); clamp 0 -> 1e-38; lx = Ln(xrep).
  2. For each K-tile t (4 nodes x 32 cin = 128 partitions) and batch b:
     F[t][b][(k,c), 66, 66] = Exp(svec * lx) into a zero-padded map
     (bf16), svec a per-partition scale vector of node values; per-batch
     tiles let each batch's matmuls start as soon as its exps land.
  3. On-device A: Lagrange basis via prefix/suffix products of (e - e_m)
     on [32, 9, 32] tiles, folded with weight, cast to bf16 into
     A_t[(k,c), ij, o].
  4. 288 matmuls: psum[o=32, 8x64] += A_t[:, :, ij].T @ F[t][b][:, rows, cols]
     accumulating 18 (t, ij) terms per output row-block; 8 PSUM banks
     cover 8 row-blocks per batch, t outermost so matmuls start after
     the first exp.
  5. Bias-add (per-partition scalar) PSUM -> SBUF; each row-block is
     DMA'd out as soon as its bias-add lands, overlapping the stores
     with the remaining matmuls.
"""

import numpy as np

import concourse.bass as bass
import concourse.mybir as mybir
import concourse.tile as tile
from concourse.bass_utils import run_bass_kernel_spmd

# ---- problem constants (hardcoded per contract) ----
B, CIN, H, W = 16, 32, 64, 64
COUT = 32
KS = 3
NCORES = 8
BPC = B // NCORES  # batches per core

# ---- interpolation constants (data-independent) ----
R = 8           # nodes
T = R // 4      # contraction tiles of (4 nodes x 32 cin) = 128


def _cheb_nodes(r):
    k = np.arange(r)
    return ((np.cos((2 * k + 1) * np.pi / (2 * r)) + 1) / 2).astype(np.float64)


_NODES = _cheb_nodes(R)
# barycentric denominators: c_k = 1 / prod_{m != k} (e_k - e_m)
_CDEN = np.array(
    [1.0 / np.prod(_NODES[k] - np.delete(_NODES, k)) for k in range(R)]
)

F32 = mybir.dt.float32
BF16 = mybir.dt.bfloat16
AF = mybir.ActivationFunctionType


def _patch_drain_wait_split():
    """The walrus in this image rejects >N sem-waits on one SP CTRL/Drain
    instruction ("Too many sync wait commands"). Split the tail drain's
    waits across single-wait NOPs on the SP engine before the drain."""
    if getattr(tile.TileContext, "_drain_split_patched", False):
        return
    from concourse.vector_clock import ScopedClock

    def _drain_and_barrier(self, tick_clock, wait_clock):
        nc = self.nc
        probe = nc.sync.nop(nofuse=True)
        wait_clock.add_sem_waits(
            probe.ins, ScopedClock({None: tick_clock.global_clock})
        )
        si = probe.ins.sync_info
        waits = list(si.on_wait) if si is not None else []
        upds = list(si.on_update) if si is not None else []
        probe.ins.sync_info = mybir.SyncInfo(on_wait=waits[:1], on_update=upds)
        for w in waits[1:]:
            nop = nc.sync.nop(nofuse=True)
            nop.ins.sync_info = mybir.SyncInfo(on_wait=[w], on_update=[])
        nc.sync.drain()

        nc.all_engine_barrier()
        assert self.sems is not None
        popped = nc._tile_sem_poison_stack.pop()
        assert popped is self._sem_poison
        nc.clear_and_free_semaphores(list(self.sems.allocated().values()))
        nc.all_engine_barrier()

    tile.TileContext._drain_and_barrier = _drain_and_barrier
    tile.TileContext._drain_split_patched = True


_patch_drain_wait_split()


def _split_excess_waits(nc, max_waits=1):
    """This image's walrus rejects instructions carrying more than ~1
    semaphore wait ("Too many sync wait commands"). Move excess waits onto
    NoOp instructions inserted just before, on the same engine (engine
    program order makes the waits happen-before the original instruction)."""
    uid = 0
    for f in nc.m.functions:
        for bb in f.blocks:
            il = bb.instructions
            i = 0
            while i < len(il):
                ins = il[i]
                si = getattr(ins, "sync_info", None)
                if si is not None and si.on_wait and len(si.on_wait) > max_waits:
                    waits = list(si.on_wait)
                    ins.sync_info = mybir.SyncInfo(
                        on_wait=waits[:max_waits], on_update=list(si.on_update)
                    )
                    nops = []
                    for w in waits[max_waits:]:
                        nop = mybir.InstNoOp(name=f"wsplit-{uid}", ins=[], outs=[])
                        uid += 1
                        nop.engine = ins.engine
                        nop.sync_info = mybir.SyncInfo(on_wait=[w], on_update=[])
                        nops.append(nop)
                    il[i:i] = nops
                    i += len(nops)
                i += 1


def build_nc():
    nc = bass.Bass("TRN2", target_bir_lowering=False, debug=False)

    x_d = nc.dram_tensor("x", [BPC, CIN, H, W], F32, kind="ExternalInput")
    w_d = nc.dram_tensor("weight", [COUT, CIN, KS, KS], F32, kind="ExternalInput")
    b_d = nc.dram_tensor("bias", [COUT], F32, kind="ExternalInput")
    e_d = nc.dram_tensor("exponent", [COUT, CIN, KS, KS], F32, kind="ExternalInput")
    o_d = nc.dram_tensor("out", [BPC, COUT, H, W], F32, kind="ExternalOutput")

    P = 128
    RB = 8           # row-blocks per batch (64 rows / 8)
    RBH = H // RB    # rows per block

    with tile.TileContext(nc) as tc:
        from contextlib import ExitStack

        with ExitStack() as ctx:
            pool = ctx.enter_context(tc.tile_pool(name="main", bufs=1))
            psum_pool = ctx.enter_context(
                tc.tile_pool(name="psum", bufs=1, space="PSUM")
            )

            # ---- per-partition node-scale vector: svec[p, t] = e_{4t + p//32}
            svec = pool.tile([P, T], F32, tag="svec")
            for t in range(T):
                for g in range(4):
                    nc.vector.memset(
                        svec[32 * g : 32 * (g + 1), t : t + 1],
                        float(_NODES[4 * t + g]),
                    )

            # ---- small inputs ----
            e_sb = pool.tile([CIN, COUT, 9], F32, tag="e_sb")
            nc.sync.dma_start(e_sb[:], e_d.ap().rearrange("o c i j -> c o (i j)"))
            w_sb = pool.tile([CIN, COUT, 9], F32, tag="w_sb")
            nc.sync.dma_start(w_sb[:], w_d.ap().rearrange("o c i j -> c o (i j)"))
            bias_sb = pool.tile([COUT, 1], F32, tag="bias_sb")
            nc.sync.dma_start(bias_sb[:], b_d.ap()[:, None])

            # ---- on-device A_t[(k,c), ij, o] = w * l_k(e) * c_k (bf16) ----
            # diffs d_m = e - e_m
            diffs = []
            for m in range(R):
                d = pool.tile([CIN, COUT, 9], F32, tag=f"d{m}")
                nc.vector.tensor_scalar_add(d[:], e_sb[:], -float(_NODES[m]))
                diffs.append(d)
            # prefix products pre[k] = prod_{m<k} d_m  (pre[0] = 1 implicit)
            pres = [None, diffs[0]]
            for k in range(2, R):
                p = pool.tile([CIN, COUT, 9], F32, tag=f"pre{k}")
                nc.vector.tensor_mul(p[:], pres[k - 1][:], diffs[k - 1][:])
                pres.append(p)
            # suffix products with weight folded in: suf[k] = w * prod_{m>k} d_m
            sufs = [None] * R
            sufs[R - 1] = w_sb
            for k in range(R - 2, -1, -1):
                s = pool.tile([CIN, COUT, 9], F32, tag=f"suf{k}")
                nc.vector.tensor_mul(s[:], sufs[k + 1][:], diffs[k + 1][:])
                sufs[k] = s

            A_ts = [
                pool.tile([P, COUT, 9], BF16, tag=f"A{t}", name=f"A{t}")
                for t in range(T)
            ]
            for k in range(R):
                t, kl = divmod(k, 4)
                dstA = A_ts[t][32 * kl : 32 * (kl + 1)]
                if k == 0:
                    tmp = sufs[0]
                else:
                    tmp = pool.tile([CIN, COUT, 9], F32, tag=f"lw{k}")
                    nc.vector.tensor_mul(tmp[:], pres[k][:], sufs[k][:])
                nc.vector.tensor_scalar_mul(dstA[:], tmp[:], float(_CDEN[k]))

            # preload the ln/exp activation table early (dummy activation)
            warmact = pool.tile([P, 1], F32, tag="warmact")
            nc.scalar.activation(warmact[:], svec[:, 0:1], AF.Exp, scale=0.0)

            # ---- x replicated over 4 partition groups, clamp, ln ----
            xrep = pool.tile([P, BPC, H, W], F32, tag="xrep")
            xsrc = x_d.ap().rearrange("b c h w -> c b h w")
            for g in range(4):
                for b in range(BPC):
                    for rh in range(2):
                        nc.sync.dma_start(
                            xrep[32 * g : 32 * (g + 1), b, rh * 32 : rh * 32 + 32],
                            xsrc[:, b, rh * 32 : rh * 32 + 32],
                        )
            lx = pool.tile([P, BPC, H, W], F32, tag="lx")
            for b in range(BPC):
                nc.vector.tensor_scalar_max(xrep[:, b], xrep[:, b], 1e-38)
                nc.scalar.activation(lx[:, b], xrep[:, b], AF.Ln)

            # ---- F[t][b] = exp(e_k * lx) into zero-padded (H+2, W+2) maps ----
            Fts = [[None] * BPC for _ in range(T)]
            for b in range(BPC):
                for t in range(T):
                    Ft = pool.tile(
                        [P, H + 2, W + 2], BF16, tag=f"F{t}_{b}", name=f"F{t}_{b}"
                    )
                    Fts[t][b] = Ft
                    nc.vector.memset(Ft[:, 0, :], 0.0)
                    nc.vector.memset(Ft[:, H + 1, :], 0.0)
                    nc.vector.memset(Ft[:, :, 0], 0.0)
                    nc.vector.memset(Ft[:, :, W + 1], 0.0)
                    nc.scalar.activation(
                        Ft[:, 1 : H + 1, 1 : W + 1],
                        lx[:, b],
                        AF.Exp,
                        scale=svec[:, t : t + 1],
                    )

            # ---- conv matmuls + bias + chunked store ----
            odst = o_d.ap().rearrange("b o h w -> o b h w")
            out_sb = pool.tile([COUT, BPC, H, W], F32, tag="out_sb")
            for half in range(BPC):
                pts = [
                    psum_pool.tile(
                        [COUT, RBH, W], F32, tag=f"pt{rb}", name=f"pt{rb}"
                    )
                    for rb in range(RB)
                ]
                for t in range(T):
                    for ij in range(9):
                        i, j = divmod(ij, 3)
                        lhsT = A_ts[t][:, :, ij]
                        for rb in range(RB):
                            r0 = rb * RBH + i
                            rhs = Fts[t][half][:, r0 : r0 + RBH, j : j + W]
                            nc.tensor.matmul(
                                pts[rb][:],
                                lhsT,
                                rhs,
                                start=(t == 0 and ij == 0),
                                stop=(t == T - 1 and ij == 8),
                            )
                for rb in range(RB):
                    nc.vector.tensor_scalar_add(
                        out_sb[:, half, rb * RBH : (rb + 1) * RBH, :],
                        pts[rb][:],
                        bias_sb[:],
                    )
                # store each row-block as soon as its bias-add lands, so
                # the stores overlap the remaining matmuls
                for rb in range(RB):
                    nc.sync.dma_start(
                        odst[:, half, rb * RBH : (rb + 1) * RBH, :],
                        out_sb[:, half, rb * RBH : (rb + 1) * RBH, :],
                    )

    _split_excess_waits(nc)
    return nc


_NC_CACHE = {}


def _get_nc():
    if "nc" not in _NC_CACHE:
        _NC_CACHE["nc"] = build_nc()
    return _NC_CACHE["nc"]


def kernel(**inputs) -> np.ndarray:
    x = np.ascontiguousarray(np.asarray(inputs["x"], dtype=np.float32))
    w = np.ascontiguousarray(np.asarray(inputs["weight"], dtype=np.float32))
    b = np.ascontiguousarray(np.asarray(inputs["bias"], dtype=np.float32))
    e = np.ascontiguousarray(np.asarray(inputs["exponent"], dtype=np.float32))

    nc = _get_nc()
    in_maps = [
        {
            "x": np.ascontiguousarray(x[i * BPC : (i + 1) * BPC]),
            "weight": w,
            "bias": b,
            "exponent": e,
        }
        for i in range(NCORES)
    ]
    res = run_bass_kernel_spmd(nc, in_maps, core_ids=list(range(NCORES)))
    out = np.concatenate(
        [np.asarray(res.results[i]["out"]) for i in range(NCORES)], axis=0
    )
    return out.astype(np.float32)

